# revision 1
# baseline (speedup 1.0000x reference)
"""Multi-head graph attention message passing on 8 Trainium2 cores.

Strategy (graph/data parallel, per the dst-sharding scheme):
  - Nodes sharded by dst across 8 cores (12500 each). Each core owns the
    wV rows for its dst range; segment_sum is local via hardware
    dma_scatter_add (CCE accumulate in the SDMA datapath).
  - Q/K/V projections: small weights replicated; every core computes the
    full K,V tables (replicated compute, no cross-core traffic) and the
    Q table for its own node range only. K,V stored interleaved per node
    row [K|V] so one dma_gather per edge fetches both.
  - Edges routed by dst partition on host; within a core, grouped by src
    chunk (4 chunks) so dma_gather int16 indices stay in range.
"""

import numpy as np

import concourse.bacc as bacc
import concourse.mybir as mybir
import concourse.tile as tile
from concourse.bass_utils import run_bass_kernel_spmd

F32 = mybir.dt.float32
I16 = mybir.dt.int16


class Cfg:
    n_nodes = 100000
    n_edges = 1600000
    in_dim = 128
    heads = 8
    hdim = 16
    hid = 128          # heads * hdim
    n_cores = 8
    n_chunks = 4       # src chunks for int16 gather indices
    batch = 1024       # edges per device batch (dma_gather caps near 1024 idxs/call)
    proj_tile = 512    # nodes per projection DMA group
    kv_bf16 = False    # store K,V tables in bf16 (halves gather traffic)

    def __init__(self, **kw):
        for k, v in kw.items():
            setattr(self, k, v)
        assert self.n_nodes % self.n_cores == 0
        self.own = self.n_nodes // self.n_cores
        # padded full node count: multiple of proj_tile and n_chunks
        m = self.proj_tile * self.n_chunks
        self.np_pad = -(-self.n_nodes // m) * m
        self.cr = self.np_pad // self.n_chunks          # chunk rows
        assert self.cr <= 32767, "gather idx must fit int16"
        self.own_pad = -(-self.own // self.proj_tile) * self.proj_tile
        self.wv_rows = self.own_pad + 128  # spare rows for the padding sink
        assert self.wv_rows <= 32767
        self.dummy_row = self.own_pad  # scatter target for padding edges


def build_program(cfg, g_pad):
    """One SPMD program; per-core behavior differs only through input data."""
    nc = bacc.Bacc("TRN2", target_bir_lowering=False, debug=False)
    W = g_pad // 16

    xt = nc.dram_tensor("xt", [cfg.in_dim, cfg.np_pad], F32, kind="ExternalInput")
    xt_own = nc.dram_tensor("xt_own", [cfg.in_dim, cfg.own_pad], F32, kind="ExternalInput")
    w_kv = nc.dram_tensor("w_kv", [cfg.in_dim, 2 * cfg.hid], F32, kind="ExternalInput")
    w_q = nc.dram_tensor("w_q", [cfg.in_dim, cfg.hid], F32, kind="ExternalInput")
    b_kv = nc.dram_tensor("b_kv", [128, 2 * cfg.hid], F32, kind="ExternalInput")
    b_q = nc.dram_tensor("b_q", [128, cfg.hid], F32, kind="ExternalInput")
    kv_idx = nc.dram_tensor("kv_idx", [cfg.n_chunks, 128, W], I16, kind="ExternalInput")
    q_idx = nc.dram_tensor("q_idx", [cfg.n_chunks, 128, W], I16, kind="ExternalInput")
    sc_idx = nc.dram_tensor("sc_idx", [cfg.n_chunks, 128, W], I16, kind="ExternalInput")

    wv = nc.dram_tensor("wv", [cfg.wv_rows, cfg.hid], F32, kind="ExternalOutput")

    KVDT = mybir.dt.bfloat16 if cfg.kv_bf16 else F32
    kv_tab = nc.dram_tensor("kv_tab", [cfg.np_pad, 2 * cfg.hid], KVDT)
    q_tab = nc.dram_tensor("q_tab", [cfg.own_pad, cfg.hid], F32)

    PT = cfg.proj_tile
    B = cfg.batch
    BC = B // 128  # column groups per batch tile

    with tile.TileContext(nc) as tc:
        with (
            tc.tile_pool(name="const", bufs=1) as cpool,
            tc.tile_pool(name="proj", bufs=3) as ppool,
            tc.tile_pool(name="psum", bufs=4, space="PSUM") as psum,
            tc.tile_pool(name="edge", bufs=3) as epool,
            tc.tile_pool(name="idx", bufs=3) as ipool,
        ):
            w_kv_t = cpool.tile([cfg.in_dim, 2 * cfg.hid], F32)
            w_q_t = cpool.tile([cfg.in_dim, cfg.hid], F32)
            b_kv_t = cpool.tile([128, 2 * cfg.hid], F32)
            b_q_t = cpool.tile([128, cfg.hid], F32)
            nc.sync.dma_start(w_kv_t[:], w_kv[:])
            nc.sync.dma_start(w_q_t[:], w_q[:])
            nc.sync.dma_start(b_kv_t[:], b_kv[:])
            nc.sync.dma_start(b_q_t[:], b_q[:])

            zt = cpool.tile([128, 4 * cfg.hid], F32)
            nc.vector.memset(zt[:], 0.0)
            for r in range(0, cfg.wv_rows, 512):
                rows = min(512, cfg.wv_rows - r)
                zview = wv[r:r + rows, :].rearrange("(s p) e -> p s e", p=128)
                nc.sync.dma_start(
                    zview, zt[:, :rows].rearrange("p (s e) -> p s e", e=cfg.hid))

            def project(src_dram, n_pad, w_t, b_t, out_dram, out_width, odt=F32):
                for g in range(n_pad // PT):
                    xt_t = ppool.tile([128, PT], F32, tag="xt_t")
                    nc.sync.dma_start(xt_t[:], src_dram[:, g * PT:(g + 1) * PT])
                    out_sb = ppool.tile([128, PT // 128, out_width], odt, tag="out_sb")
                    for s in range(PT // 128):
                        ps = psum.tile([128, out_width], F32)
                        nc.tensor.matmul(
                            ps[:], xt_t[:, s * 128:(s + 1) * 128], w_t[:],
                            start=True, stop=True,
                        )
                        nc.vector.tensor_add(out_sb[:, s, :], ps[:], b_t[:])
                    dview = out_dram[g * PT:(g + 1) * PT, :].rearrange(
                        "(s p) e -> p s e", p=128)
                    nc.sync.dma_start(dview, out_sb[:])

            project(xt, cfg.np_pad, w_kv_t, b_kv_t, kv_tab, 2 * cfg.hid, KVDT)
            project(xt_own, cfg.own_pad, w_q_t, b_q_t, q_tab, cfg.hid)

            for ch in range(cfg.n_chunks):
                kv_src = kv_tab[ch * cfg.cr:(ch + 1) * cfg.cr, :]
                for b in range(g_pad // B):
                    c0 = b * (B // 16)
                    kvi = ipool.tile([128, B // 16], I16, tag="kvi")
                    qi = ipool.tile([128, B // 16], I16, tag="qi")
                    sci = ipool.tile([128, B // 16], I16, tag="sci")
                    nc.sync.dma_start(kvi[:], kv_idx[ch, :, c0:c0 + B // 16])
                    nc.sync.dma_start(qi[:], q_idx[ch, :, c0:c0 + B // 16])
                    nc.sync.dma_start(sci[:], sc_idx[ch, :, c0:c0 + B // 16])

                    kv_t = epool.tile([128, BC, 2 * cfg.hid], KVDT, tag="kv_t")
                    q_t = epool.tile([128, BC, cfg.hid], F32, tag="q_t")
                    nc.gpsimd.dma_gather(
                        kv_t[:], kv_src, kvi[:], B, B, 2 * cfg.hid)
                    nc.gpsimd.dma_gather(
                        q_t[:], q_tab[:], qi[:], B, B, cfg.hid)

                    prod = epool.tile([128, BC, cfg.hid], F32, tag="prod")
                    nc.vector.tensor_mul(prod[:], kv_t[:, :, :cfg.hid], q_t[:])

                    sc = epool.tile([128, BC, cfg.heads], F32, tag="sc")
                    nc.vector.reduce_sum(
                        sc[:],
                        prod[:].rearrange("p c (h d) -> p c h d", d=cfg.hdim),
                        axis=mybir.AxisListType.X,
                    )
                    # clip(dot/scale, ±5) == clip(dot, ±5*scale) then /scale
                    lim = 5.0 * float(np.sqrt(cfg.hdim))
                    nc.vector.tensor_scalar_min(sc[:], sc[:], lim)
                    nc.vector.tensor_scalar_max(sc[:], sc[:], -lim)
                    ex = epool.tile([128, BC, cfg.heads], F32, tag="ex")
                    nc.scalar.activation(
                        ex[:], sc[:], mybir.ActivationFunctionType.Exp,
                        scale=float(1.0 / np.sqrt(cfg.hdim)),
                    )

                    msg = epool.tile([128, BC, cfg.hid], F32, tag="msg")
                    nc.vector.tensor_mul(
                        msg[:].rearrange("p c (h d) -> p c h d", d=cfg.hdim),
                        kv_t[:, :, cfg.hid:].rearrange(
                            "p c (h d) -> p c h d", d=cfg.hdim),
                        ex[:].unsqueeze(-1).broadcast_to(
                            [128, BC, cfg.heads, cfg.hdim]),
                    )
                    nc.gpsimd.dma_scatter_add(
                        wv[:], msg[:], sci[:], B, B, cfg.hid)
    nc.finalize()
    return nc


def _wrap16(a, g_pad):
    """[n] -> [128, g_pad//16] int16: idx i at [i%16 (+16k replicas), i//16]."""
    w = a.reshape(g_pad // 16, 16).T.astype(np.int16)  # [16, W]
    return np.tile(w, (8, 1))


def _schedule_batches(dst_local, batch):
    """Assign edges to batches of size `batch` so that no dst row appears
    twice within one batch (dma_scatter_add RMW races on duplicate rows
    within a single call). Returns (n_batches, edge order as an index
    array grouped by batch, per-batch counts)."""
    cnt = len(dst_local)
    if cnt == 0:
        return 1, np.empty(0, np.int64), np.zeros(1, np.int64)
    order = np.argsort(dst_local, kind="stable")
    uniq, starts, degs = np.unique(
        dst_local[order], return_index=True, return_counts=True)
    nb = max(-(-cnt // batch), int(degs.max()))
    big_first = np.argsort(-degs, kind="stable")
    while True:
        fills = np.zeros(nb, np.int64)
        bin_of = np.empty(cnt, np.int64)
        ok = True
        for gi in big_first:
            d = degs[gi]
            cand = np.argsort(fills, kind="stable")[:d]
            if fills[cand[-1]] >= batch:
                ok = False
                break
            fills[cand] += 1
            s = starts[gi]
            bin_of[order[s:s + d]] = cand
        if ok:
            break
        nb += 1
    batch_order = np.argsort(bin_of, kind="stable")
    counts = np.bincount(bin_of, minlength=nb)
    return nb, batch_order, counts


def prepare_inputs(cfg, x, src, dst, Wq, bq, Wk, bk, Wv, bv):
    x = np.asarray(x, np.float32)
    src = np.asarray(src, np.int64)
    dst = np.asarray(dst, np.int64)

    xt = np.zeros((cfg.in_dim, cfg.np_pad), np.float32)
    xt[:, :cfg.n_nodes] = x.T
    w_kv = np.concatenate([np.asarray(Wk, np.float32),
                           np.asarray(Wv, np.float32)], axis=1)
    b_kv = np.tile(np.concatenate([np.asarray(bk, np.float32),
                                   np.asarray(bv, np.float32)])[None, :], (128, 1))
    w_q = np.asarray(Wq, np.float32)
    b_q = np.tile(np.asarray(bq, np.float32)[None, :], (128, 1))

    core_of = dst // cfg.own
    chunk_of = src // cfg.cr

    # per-(core, chunk) edge lists, scheduled into duplicate-free batches
    groups = {}
    nb_max = 1
    for c in range(cfg.n_cores):
        in_c = np.nonzero(core_of == c)[0]
        ch_c = chunk_of[in_c]
        for ch in range(cfg.n_chunks):
            e = in_c[ch_c == ch]
            nb, border, counts = _schedule_batches(
                (dst[e] - c * cfg.own), cfg.batch)
            groups[(c, ch)] = (e[border] if len(e) else e, counts)
            nb_max = max(nb_max, nb)
    g_pad = nb_max * cfg.batch

    in_maps = []
    for c in range(cfg.n_cores):
        kvi = np.zeros((cfg.n_chunks, 128, g_pad // 16), np.int16)
        qi = np.zeros((cfg.n_chunks, 128, g_pad // 16), np.int16)
        sci = np.zeros((cfg.n_chunks, 128, g_pad // 16), np.int16)
        for ch in range(cfg.n_chunks):
            e, counts = groups[(c, ch)]
            kv_l = np.zeros(g_pad, np.int64)
            q_l = np.zeros(g_pad, np.int64)
            sc_l = np.full(g_pad, cfg.dummy_row, np.int64)
            pos = 0
            off = 0
            for b, cnt in enumerate(counts):
                eb = e[pos:pos + cnt]
                kv_l[off:off + cnt] = src[eb] - ch * cfg.cr
                q_l[off:off + cnt] = dst[eb] - c * cfg.own
                sc_l[off:off + cnt] = dst[eb] - c * cfg.own
                pos += cnt
                off += cfg.batch
            kvi[ch] = _wrap16(kv_l, g_pad)
            qi[ch] = _wrap16(q_l, g_pad)
            sci[ch] = _wrap16(sc_l, g_pad)

        xt_own = np.zeros((cfg.in_dim, cfg.own_pad), np.float32)
        xt_own[:, :cfg.own] = x[c * cfg.own:(c + 1) * cfg.own].T
        in_maps.append({
            "xt": xt, "xt_own": xt_own,
            "w_kv": w_kv, "w_q": w_q, "b_kv": b_kv, "b_q": b_q,
            "kv_idx": kvi, "q_idx": qi, "sc_idx": sci,
        })
    return in_maps, g_pad


def kernel(x, src, dst, Wq, bq, Wk, bk, Wv, bv):
    cfg = Cfg()
    in_maps, g_pad = prepare_inputs(cfg, x, src, dst, Wq, bq, Wk, bk, Wv, bv)
    nc = build_program(cfg, g_pad)
    res = run_bass_kernel_spmd(nc, in_maps, list(range(cfg.n_cores)))
    out = np.concatenate(
        [res.results[c]["wv"][:cfg.own] for c in range(cfg.n_cores)], axis=0)
    return out.reshape(cfg.n_nodes, cfg.heads, cfg.hdim)



# revision 7
# speedup vs baseline: 3.9973x; 3.9973x over previous
"""Multi-head graph attention message passing on 8 Trainium2 cores.

Design (v2, dst-stationary — eliminates the Q-gather and the scatter-add):
  - Nodes sharded by dst across 8 cores (12500 each).
  - Per core and per src-chunk (4 chunks of 25600 nodes, so gather indices
    fit int16), the core's dst nodes are sorted by their degree *within that
    chunk* and grouped into blocks of 128 (one dst node per SBUF partition).
    Block width = max degree in block (tight, since degree-sorted).
  - One dma_gather per group of blocks fetches KV rows (bf16, 512B) for all
    edges; slot (p, s) holds the s-th edge of the block's p-th dst node.
    Padding slots point at a zeroed table row (V=0 => contributes nothing).
  - Q is never gathered: the host pre-permutes x into per-chunk dst order,
    and Q is projected per block straight into SBUF (matmul + bias).
  - score = exp(clip(sum_d K*Q / 4)) computed on DVE+ACT; messages reduced
    over slots per partition; per-chunk partial wV written contiguously.
  - Host sums the 4 permuted partials (inverse permutation) at the end.
  SWDGE descriptor generation is the bottleneck (~9 ns/descriptor on the Q7);
  this design needs ~1 descriptor per edge (+ ~10% padding) and nothing else.
"""

import numpy as np
import ml_dtypes

import concourse.bacc as bacc
import concourse.mybir as mybir
import concourse.tile as tile
from concourse.bass_utils import run_bass_kernel_spmd

F32 = mybir.dt.float32
BF16 = mybir.dt.bfloat16
I16 = mybir.dt.int16


class Cfg:
    n_nodes = 100000
    n_edges = 1600000
    in_dim = 128
    heads = 8
    hdim = 16
    hid = 128            # heads * hdim
    n_cores = 8
    n_chunks = 4         # src chunks so int16 gather indices stay in range
    proj_tile = 512      # nodes per projection matmul group
    group_slots = 30     # target gather-call size in slots (x128 = num_idxs);
                         # keep num_idxs <= 4096 per call (ring capacity)
    n_queues = 4         # SWDGE queues to rotate gathers over
    dma_scratch = 16384  # descriptor-ring carveout bytes per partition
    clip_margin = 19.5   # |score| beyond this forces on-device clamping

    def __init__(self, **kw):
        for k, v in kw.items():
            setattr(self, k, v)
        assert self.n_nodes % self.n_cores == 0
        self.own = self.n_nodes // self.n_cores          # 12500
        self.own_pad = -(-self.own // 128) * 128         # 12544
        self.n_blocks = self.own_pad // 128              # 98
        self.cr = 25600                                  # chunk rows
        assert self.cr * self.n_chunks >= self.n_nodes
        self.tab_rows = self.cr + 128                    # + zero pad rows
        self.np_pad = self.cr * self.n_chunks            # 102400
        self.dummy_row = self.cr                         # chunk-local zero row


def build_program(cfg, plan):
    """One SPMD program; per-core behavior differs only through input data."""
    nc = bacc.Bacc("TRN2", target_bir_lowering=False, debug=False,
                   num_swdge_queues=cfg.n_queues,
                   dynamic_dma_scratch_size=cfg.dma_scratch)

    xt = nc.dram_tensor("xt", [cfg.in_dim, cfg.np_pad], F32, kind="ExternalInput")
    xq = nc.dram_tensor("xq", [cfg.n_chunks, cfg.in_dim, cfg.own_pad], F32,
                        kind="ExternalInput")
    w_kv = nc.dram_tensor("w_kv", [cfg.in_dim, 2 * cfg.hid], F32, kind="ExternalInput")
    w_q = nc.dram_tensor("w_q", [cfg.in_dim, cfg.hid], F32, kind="ExternalInput")
    b_kv = nc.dram_tensor("b_kv", [128, 2 * cfg.hid], F32, kind="ExternalInput")
    b_q = nc.dram_tensor("b_q", [128, cfg.hid], F32, kind="ExternalInput")
    eidx = nc.dram_tensor("eidx", [128, plan["total_cols"]], I16,
                          kind="ExternalInput")
    wv = nc.dram_tensor("wv", [cfg.n_chunks, cfg.own_pad, cfg.hid], F32,
                        kind="ExternalOutput")

    kv_tabs = [nc.dram_tensor(f"kv_tab{c}", [cfg.tab_rows, 2 * cfg.hid], BF16)
               for c in range(cfg.n_chunks)]

    PT = cfg.proj_tile
    scale = float(np.sqrt(cfg.hdim))
    lim = 5.0 * scale

    with tile.TileContext(nc) as tc:
        with (
            tc.tile_pool(name="const", bufs=1) as cpool,
            tc.tile_pool(name="proj", bufs=3) as ppool,
            tc.tile_pool(name="psum", bufs=4, space="PSUM") as psum,
            tc.tile_pool(name="qsum", bufs=4, space="PSUM") as qsum,
            tc.tile_pool(name="edge", bufs=3) as epool,
            tc.tile_pool(name="idx", bufs=3) as ipool,
            tc.tile_pool(name="blk", bufs=3) as bpool,
            tc.tile_pool(name="out", bufs=3) as opool,
        ):
            w_kv_t = cpool.tile([cfg.in_dim, 2 * cfg.hid], F32)
            w_q_t = cpool.tile([cfg.in_dim, cfg.hid], F32)
            b_kv_t = cpool.tile([128, 2 * cfg.hid], F32)
            b_q_t = cpool.tile([128, cfg.hid], F32)
            nc.sync.dma_start(w_kv_t[:], w_kv[:])
            nc.sync.dma_start(w_q_t[:], w_q[:])
            nc.sync.dma_start(b_kv_t[:], b_kv[:])
            nc.sync.dma_start(b_q_t[:], b_q[:])

            # zero pad rows of each chunk table (gather target for padding)
            zt = cpool.tile([128, 2 * cfg.hid], BF16)
            nc.vector.memset(zt[:], 0.0)
            for ch in range(cfg.n_chunks):
                nc.sync.dma_start(kv_tabs[ch][cfg.cr:cfg.tab_rows, :], zt[:])

            # K|V projection, chunk by chunk (gathers of chunk ch wait only
            # on chunk ch's table writes)
            for ch in range(cfg.n_chunks):
                for g in range(cfg.cr // PT):
                    c0 = ch * cfg.cr + g * PT
                    xt_t = ppool.tile([128, PT], F32, tag="xt_t")
                    nc.sync.dma_start(xt_t[:], xt[:, c0:c0 + PT])
                    out_sb = ppool.tile([128, PT // 128, 2 * cfg.hid], BF16,
                                        tag="out_sb")
                    for s in range(PT // 128):
                        ps = psum.tile([128, 2 * cfg.hid], F32)
                        nc.tensor.matmul(
                            ps[:], xt_t[:, s * 128:(s + 1) * 128], w_kv_t[:],
                            start=True, stop=True)
                        nc.vector.tensor_add(out_sb[:, s, :], ps[:], b_kv_t[:])
                    dview = kv_tabs[ch][g * PT:(g + 1) * PT, :].rearrange(
                        "(s p) e -> p s e", p=128)
                    nc.sync.dma_start(dview, out_sb[:])

            # edge phase: one gather per group of dst blocks
            for (ch, b0, ws, col0, S) in plan["groups"]:
                n_idx = S * 128
                it = ipool.tile([128, n_idx // 16], I16, tag="it")
                nc.sync.dma_start(it[:], eidx[:, col0:col0 + n_idx // 16])
                gt = epool.tile([128, S, 2 * cfg.hid], BF16, tag="gt")
                nc.gpsimd.dma_gather(
                    gt[:], kv_tabs[ch][:], it[:], n_idx, n_idx, 2 * cfg.hid,
                    queue_num=plan["gq"][(ch, b0)] % cfg.n_queues,
                    single_packet=n_idx <= 1024)

                nb = len(ws)
                xq_t = bpool.tile([128, nb * 128], F32, tag="xq_t")
                nc.sync.dma_start(
                    xq_t[:], xq[ch, :, b0 * 128:(b0 + nb) * 128])
                ov = opool.tile([128, nb, cfg.hid], F32, tag="ov")

                off = 0
                for k, W in enumerate(ws):
                    qp = qsum.tile([128, cfg.hid], F32)
                    nc.tensor.matmul(
                        qp[:], xq_t[:, k * 128:(k + 1) * 128], w_q_t[:],
                        start=True, stop=True)
                    qb = bpool.tile([128, cfg.hid], BF16, tag="qb")
                    nc.vector.tensor_add(qb[:], qp[:], b_q_t[:])

                    kvb = gt[:, off:off + W, :]
                    prod = bpool.tile([128, W, cfg.hid], BF16, tag="prod")
                    nc.vector.tensor_mul(
                        prod[:], kvb[:, :, :cfg.hid],
                        qb[:].unsqueeze(1).broadcast_to([128, W, cfg.hid]))
                    sc = bpool.tile([128, W, cfg.heads], F32, tag="sc")
                    nc.vector.reduce_sum(
                        sc[:],
                        prod[:].rearrange("p w (h d) -> p w h d", d=cfg.hdim),
                        axis=mybir.AxisListType.X)
                    if plan["need_clip"]:
                        nc.vector.tensor_scalar_min(sc[:], sc[:], lim)
                        nc.vector.tensor_scalar_max(sc[:], sc[:], -lim)
                    ex = bpool.tile([128, W, cfg.heads], BF16, tag="ex")
                    nc.scalar.activation(
                        ex[:], sc[:], mybir.ActivationFunctionType.Exp,
                        scale=float(1.0 / scale))

                    msg = bpool.tile([128, W, cfg.hid], F32, tag="msg")
                    nc.vector.tensor_mul(
                        msg[:].rearrange("p w (h d) -> p w h d", d=cfg.hdim),
                        kvb[:, :, cfg.hid:].rearrange(
                            "p w (h d) -> p w h d", d=cfg.hdim),
                        ex[:].unsqueeze(-1).broadcast_to(
                            [128, W, cfg.heads, cfg.hdim]))
                    # reduce over slots
                    if W == 1:
                        nc.vector.tensor_scalar_add(ov[:, k, :], msg[:, 0, :], 0.0)
                    else:
                        nc.vector.tensor_add(ov[:, k, :], msg[:, 0, :],
                                             msg[:, 1, :])
                        for s in range(2, W):
                            nc.vector.tensor_add(ov[:, k, :], ov[:, k, :],
                                                 msg[:, s, :])
                    off += W

                dview = wv[ch, b0 * 128:(b0 + nb) * 128, :].rearrange(
                    "(s p) e -> p s e", p=128)
                nc.sync.dma_start(dview, ov[:])
    nc.finalize()
    return nc


def _wrap16(a):
    """[n] -> [128, n//16] int16: idx i at [i%16 (+16k replicas), i//16]."""
    w = a.reshape(-1, 16).T.astype(np.int16)
    return np.tile(w, (8, 1))


def prepare_inputs(cfg, x, src, dst, Wq, bq, Wk, bk, Wv, bv):
    x = np.asarray(x, np.float32)
    src = np.asarray(src, np.int64)
    dst = np.asarray(dst, np.int64)

    xt = np.zeros((cfg.in_dim, cfg.np_pad), np.float32)
    xt[:, :cfg.n_nodes] = x.T
    w_kv = np.concatenate([np.asarray(Wk, np.float32),
                           np.asarray(Wv, np.float32)], axis=1)
    b_kv = np.tile(np.concatenate([np.asarray(bk, np.float32),
                                   np.asarray(bv, np.float32)])[None, :], (128, 1))
    w_q = np.asarray(Wq, np.float32)
    b_q = np.tile(np.asarray(bq, np.float32)[None, :], (128, 1))

    core_of = dst // cfg.own
    chunk_of = src // cfg.cr

    # per (core, chunk): degree-sort dst nodes, assign edge slots
    perms = np.empty((cfg.n_cores, cfg.n_chunks, cfg.own_pad), np.int64)
    ew = {}   # (c, ch) -> (flat slot positions, chunk-local src values)
    wcc = np.zeros((cfg.n_cores, cfg.n_chunks, cfg.n_blocks), np.int64)
    for c in range(cfg.n_cores):
        in_c = np.nonzero(core_of == c)[0]
        ch_all = chunk_of[in_c]
        for ch in range(cfg.n_chunks):
            e = in_c[ch_all == ch]
            dl = dst[e] - c * cfg.own
            sl = src[e] - ch * cfg.cr
            cnt = np.bincount(dl, minlength=cfg.own_pad)
            perm = np.argsort(-cnt, kind="stable")
            perms[c, ch] = perm
            scnt = cnt[perm]
            wcc[c, ch] = scnt[::128][:cfg.n_blocks]
            rank = np.empty(cfg.own_pad, np.int64)
            rank[perm] = np.arange(cfg.own_pad)
            r = rank[dl]
            o = np.argsort(r, kind="stable")
            rs, sls = r[o], sl[o]
            starts = np.cumsum(scnt) - scnt
            pos = np.arange(len(e)) - starts[rs]
            ew[(c, ch)] = (rs, pos, sls)

    W = np.maximum(wcc.max(axis=0), 1)  # [n_chunks, n_blocks], shared program

    # group blocks into gather calls
    groups = []
    gq = {}
    col = 0
    qn = 0
    slot_off = np.zeros((cfg.n_chunks, cfg.n_blocks), np.int64)
    for ch in range(cfg.n_chunks):
        b = 0
        while b < cfg.n_blocks:
            ws = []
            S = 0
            b0 = b
            while b < cfg.n_blocks and (not ws or S + W[ch, b] <= cfg.group_slots):
                slot_off[ch, b] = S
                ws.append(int(W[ch, b]))
                S += int(W[ch, b])
                b += 1
            groups.append((ch, b0, ws, col, S))
            gq[(ch, b0)] = qn
            qn += 1
            col += S * 128 // 16
    total_cols = col

    # block base slot within its phase-flat index array
    base = {}
    for (ch, b0, ws, col0, S) in groups:
        for k, w in enumerate(ws):
            base[(ch, b0 + k)] = (col0 * 16 // 128, slot_off[ch, b0 + k])

    # per-core edge index arrays
    in_maps = []
    for c in range(cfg.n_cores):
        flat = np.full(total_cols * 16, cfg.dummy_row, np.int64)
        for ch in range(cfg.n_chunks):
            rs, pos, sls = ew[(c, ch)]
            b = rs // 128
            p = rs % 128
            gbase = np.empty(cfg.n_blocks, np.int64)
            for bb in range(cfg.n_blocks):
                g0, so = base[(ch, bb)]
                gbase[bb] = g0 + so
            fp = (gbase[b] + pos) * 128 + p
            flat[fp] = sls
        eidx = _wrap16(flat)

        xq = np.zeros((cfg.n_chunks, cfg.in_dim, cfg.own_pad), np.float32)
        x_loc = x[c * cfg.own:(c + 1) * cfg.own]
        for ch in range(cfg.n_chunks):
            pm = perms[c, ch]
            xp = x_loc[np.clip(pm, 0, cfg.own - 1)]
            xp[pm >= cfg.own] = 0.0
            xq[ch] = xp.T
        in_maps.append({
            "xt": xt, "xq": xq,
            "w_kv": w_kv, "w_q": w_q, "b_kv": b_kv, "b_q": b_q,
            "eidx": eidx,
        })

    # does the data ever reach the clip boundary?
    K = (x @ np.asarray(Wk, np.float32) + np.asarray(bk, np.float32))
    Q = (x @ np.asarray(Wq, np.float32) + np.asarray(bq, np.float32))
    Kh = K.reshape(cfg.n_nodes, cfg.heads, cfg.hdim)
    Qh = Q.reshape(cfg.n_nodes, cfg.heads, cfg.hdim)
    mx = 0.0
    for a in range(0, cfg.n_edges, 200000):
        b_ = min(a + 200000, cfg.n_edges)
        d = np.einsum("ehd,ehd->eh", Kh[src[a:b_]], Qh[dst[a:b_]])
        mx = max(mx, float(np.abs(d).max()))
    need_clip = mx >= cfg.clip_margin

    plan = {"groups": groups, "gq": gq, "total_cols": total_cols,
            "need_clip": need_clip, "max_score": mx}
    return in_maps, plan, perms


def postprocess(cfg, results, perms):
    outs = []
    for c in range(cfg.n_cores):
        wv = results[c]["wv"]  # [n_chunks, own_pad, hid]
        acc = np.zeros((cfg.own_pad, cfg.hid), np.float32)
        for ch in range(cfg.n_chunks):
            acc[perms[c, ch]] += wv[ch]
        outs.append(acc[:cfg.own])
    out = np.concatenate(outs, axis=0)
    return out.reshape(cfg.n_nodes, cfg.heads, cfg.hdim)


def kernel(x, src, dst, Wq, bq, Wk, bk, Wv, bv):
    cfg = Cfg()
    in_maps, plan, perms = prepare_inputs(
        cfg, x, src, dst, Wq, bq, Wk, bk, Wv, bv)
    nc = build_program(cfg, plan)
    res = run_bass_kernel_spmd(nc, in_maps, list(range(cfg.n_cores)))
    return postprocess(cfg, res.results, perms)
